# revision 33
# baseline (speedup 1.0000x reference)
"""Trainium2 Bass kernel for nn_AutoEncoder (segment_reduce).

Self-contained: hardcodes all shapes. 8-core data-parallel over sets.

Per-core pipeline (Bc=8192 sets, Nc=65536 elems), chunked by 1024 sets.
No GPSIMD instructions (this runtime lacks the extended-instruction ucode).
  interleaved x load (row g -> partition g%128) -> mag = x@rank_w (DVE, f32)
  -> within-set stable ranks via PE shift-matrix matmuls + DVE masked
  compares -> per-tile one-hot permutation P (DVE is_equal, bf16) -> sort +
  transpose fused in one PE matmul per 128 rows (xsT = x_tile.T @ P) -> psi
  MLP with the positional-encoding multiply folded into 8 PSUM-accumulated
  matmuls (= segment sum for free) -> phi (pos_n one-hot folded to bias) ->
  size-pred/argmax (PE transposes + DVE max_index) -> decoder emitting
  natural-layout rows via stationary-U matmuls (bias via ones-row of U) ->
  length-mask fused into the PSUM->SBUF copy -> 2KB-contiguous DMA out.
DVE runs phase-A of chunk c+1 skewed ahead of phase-B of chunk c.
"""
import sys
import numpy as np

sys.path.insert(0, '/opt/trn_rl_repo')

B = 65536
NPER = 8
N = B * NPER
DIM = 64
HID = 128
MAXN = 16
NCORES = 8
BC = B // NCORES          # 8192 sets per core
NC = BC * NPER            # 65536 elements per core

CH_S = 1024               # sets per chunk
CH_E = CH_S * NPER        # 8192 elements per chunk
R = CH_E // 128           # 64 rows per partition per chunk
SL = R // NPER            # 8 sets per partition per chunk
SS = 512                  # sets per phase-B subchunk
NSS = CH_S // SS          # 2

MID_PSI = 96
MID_SP = 72
MID_DEC = 96


def _mlp_np(x, w1, b1, w2, b2):
    return np.maximum(x @ w1 + b1, 0.0) @ w2 + b2


def host_prep(params):
    """Fold tiny-MLP/constant algebra on host. Returns dict of numpy consts."""
    import ml_dtypes
    p = {k: np.asarray(v, dtype=np.float32) for k, v in params.items()}
    I = np.eye(MAXN, dtype=np.float32)
    pe_seg = _mlp_np(I[:NPER], p['epe_w1'], p['epe_b1'], p['epe_w2'], p['epe_b2'])
    pe_all = _mlp_np(I, p['dpe_w1'], p['dpe_b1'], p['dpe_w2'], p['dpe_b2'])

    bf = lambda a: np.ascontiguousarray(np.asarray(a, np.float32)).astype(ml_dtypes.bfloat16)
    f32 = lambda a: np.ascontiguousarray(np.asarray(a, np.float32))

    c = {}
    w1p = np.zeros((128, MID_PSI), np.float32)
    w1p[:DIM] = p['psi_w1']
    c['w1p'] = bf(w1p)
    c['b1'] = f32(p['psi_b1'][:, None])
    c['W2'] = bf(np.concatenate([p['psi_w2'] * pe_seg[k][None, :] for k in range(NPER)], axis=1))
    c['c2'] = f32((p['psi_b2'] * pe_seg.sum(0))[:, None])
    pw1 = p['phi_w1'][:HID]
    pb1 = p['phi_b1'] + p['phi_w1'][HID + NPER]
    c['pw1a'] = bf(pw1[:, :128])
    c['pw1b'] = bf(pw1[:, 128:])
    c['pb1a'] = f32(pb1[:128, None])
    c['pb1b'] = f32(pb1[128:, None])
    c['pw2a'] = bf(p['phi_w2'][:128])
    c['pw2b'] = bf(p['phi_w2'][128:])
    c['pb2'] = f32(p['phi_b2'][:, None])
    c['spw1'] = bf(p['sp_w1'])
    c['spb1'] = f32(p['sp_b1'][:, None])
    c['spw2'] = bf(p['sp_w2'])
    c['spb2'] = f32(p['sp_b2'][:, None])
    c['D1'] = bf(np.concatenate([p['dec_w1'] * pe_all[k][:, None] for k in range(MAXN)], axis=1))
    c['db1'] = f32(p['dec_b1'][:, None])
    w2e = np.zeros((MID_DEC + 1, DIM), np.float32)
    w2e[:MID_DEC] = p['dec_w2']
    w2e[MID_DEC] = p['dec_b2']
    c['w2e'] = bf(w2e)
    c['rankw'] = f32(np.tile(p['rank_w'].reshape(1, DIM), (128, 1)))
    c['eye16'] = f32(np.eye(16, dtype=np.float32))
    c['eye128'] = f32(np.eye(128, dtype=np.float32))
    c['iotamod'] = f32(np.tile((np.arange(64) % 16).astype(np.float32)[None, :], (128, 1)))
    # shift matrices for within-set rank compares (interleaved row space):
    # ShD_s: out[rho] = mag[rho-s] if rho%8>=s else 0 ; ShU_s: out[rho]=mag[rho+s]
    SH = np.zeros((128, 14 * 128), np.float32)
    for s in range(1, NPER):
        shd = np.zeros((128, 128), np.float32)
        shu = np.zeros((128, 128), np.float32)
        for rho in range(128):
            shd[rho, rho] = 1.0
            shu[rho, rho] = 1.0
            if rho % 8 >= s:
                shd[rho - s, rho] = -1.0
            if rho % 8 <= 7 - s:
                shu[rho + s, rho] = -1.0
        SH[:, (2 * (s - 1)) * 128:(2 * (s - 1) + 1) * 128] = shd
        SH[:, (2 * (s - 1) + 1) * 128:(2 * s) * 128] = shu
    c['SH'] = f32(SH)
    MSK = np.zeros((128, 14), np.float32)
    for s in range(1, NPER):
        MSK[:, 2 * (s - 1)] = (np.arange(128) % 8 >= s).astype(np.float32)
        MSK[:, 2 * (s - 1) + 1] = (np.arange(128) % 8 <= 7 - s).astype(np.float32)
    c['MSK'] = MSK
    c['voff'] = f32((8.0 * (np.arange(128) // 8)).astype(np.float32)[:, None])
    c['iota128b'] = bf(np.tile(np.arange(128, dtype=np.float32)[None, :], (128, 1)))
    return c


CONST_SPECS = [
    ('w1p', [128, MID_PSI], 'bf16'), ('b1', [MID_PSI, 1], 'f32'),
    ('W2', [MID_PSI, NPER * 128], 'bf16'), ('c2', [128, 1], 'f32'),
    ('pw1a', [128, 128], 'bf16'), ('pw1b', [128, 8], 'bf16'),
    ('pb1a', [128, 1], 'f32'), ('pb1b', [8, 1], 'f32'),
    ('pw2a', [128, 128], 'bf16'), ('pw2b', [8, 128], 'bf16'), ('pb2', [128, 1], 'f32'),
    ('spw1', [128, MID_SP], 'bf16'), ('spb1', [MID_SP, 1], 'f32'),
    ('spw2', [MID_SP, MAXN], 'bf16'), ('spb2', [MAXN, 1], 'f32'),
    ('D1', [128, MAXN * MID_DEC], 'bf16'), ('db1', [MID_DEC, 1], 'f32'),
    ('w2e', [MID_DEC + 1, DIM], 'bf16'),
    ('rankw', [128, DIM], 'f32'),
    ('eye16', [16, 16], 'f32'), ('eye128', [128, 128], 'f32'),
    ('iotamod', [128, 64], 'f32'),
    ('SH', [128, 14 * 128], 'f32'), ('MSK', [128, 14], 'f32'),
    ('voff', [128, 1], 'f32'), ('iota128b', [128, 128], 'bf16'),
]


def build(nc, n_chunk=BC // CH_S):
    """Emit the full per-core program. n_chunk scales the problem (sim uses 1)."""
    import concourse.mybir as mybir
    from concourse import library_config
    from contextlib import ExitStack

    F32 = mybir.dt.float32
    BF16 = mybir.dt.bfloat16
    I16d = mybir.dt.int16
    I32d = mybir.dt.int32
    U32d = mybir.dt.uint32
    ALU = mybir.AluOpType
    ACTF = mybir.ActivationFunctionType
    AXX = mybir.AxisListType.X

    n_sets = n_chunk * CH_S
    n_el = n_sets * NPER
    n_sub = n_chunk * NSS
    TCNT = n_chunk * NSS * 4          # ndec column count (seq windows)

    x_ext = nc.declare_dram_parameter("x", [n_el, DIM], F32, isOutput=False)
    xr_ext = nc.declare_dram_parameter("xr", [n_sets * MAXN, DIM], F32, isOutput=True)
    nd_ext = nc.declare_dram_parameter("nd", [n_sets], I32d, isOutput=True)
    cext = {}
    for nm, shp, dt in CONST_SPECS:
        cext[nm] = nc.declare_dram_parameter(nm, shp, BF16 if dt == 'bf16' else F32,
                                             isOutput=False)

    es = ExitStack()
    sb = lambda shape, dt, name: es.enter_context(nc.sbuf_tensor(name, shape, dt))
    psu = lambda name: es.enter_context(nc.psum_tensor(name, [128, 512], F32))
    sem = lambda name: es.enter_context(nc.semaphore(name))

    C = {}
    for nm, shp, dt in CONST_SPECS:
        C[nm] = sb(shp, BF16 if dt == 'bf16' else F32, f"c_{nm}")
    xf = [sb([128, R * DIM], F32, f"xf{b}") for b in range(2)]
    xb16 = [sb([128, R * DIM], BF16, f"xb16_{b}") for b in range(2)]
    Pb0 = sb([128, R * 128], BF16, "Pb0")
    xsT = [sb([64, CH_E], BF16, f"xsT{b}") for b in range(2)]
    hsT = [sb([MID_PSI, CH_E], BF16, f"hsT{b}") for b in range(2)]
    scr = sb([128, 16 * DIM], F32, "scr")
    mag = sb([128, R], F32, "mag")
    shb = sb([128, 14 * R], F32, "shb")
    cmpb = sb([128, R * 16], F32, "cmpb")
    rank = sb([128, R], F32, "rank")
    vv = sb([128, R], F32, "vv")
    y2sb = [sb([128, SS], BF16, f"y2sb{b}") for b in range(2)]
    z1asb = sb([128, SS], BF16, "z1asb")
    z1bsb = sb([8, SS], BF16, "z1bsb")
    zsb = [sb([128, SS], BF16, f"zsb{b}") for b in range(2)]
    s1sb = sb([MID_SP, SS], BF16, "s1sb")
    npsb = sb([MAXN, SS], F32, "npsb")
    npT = sb([128, 128], F32, "npT")
    mx8 = sb([128, 64], F32, "mx8")
    am8 = sb([128, 64], U32d, "am8")
    ndf = sb([128, TCNT], F32, "ndf")
    ndQ = sb([128, 4], F32, "ndQ")
    mrow = sb([128, 64], F32, "mrow")
    U = [sb([128, SS * MAXN], BF16, f"U{b}") for b in range(2)]
    outsb = [sb([128, 512], F32, f"outsb{b}") for b in range(4)]
    ndi = sb([TCNT, 128], I32d, "ndi")

    PS = [psu(f"PS{i}") for i in range(8)]

    sCONST = sem("sCONST")
    sX = [sem("sX0"), sem("sX1")]
    sOUT = [sem(f"sOUT{j}") for j in range(4)]
    sOUTND = sem("sOUTND")
    sMS = sem("sMS")
    sMAG = sem("sMAG")    # DVE: mag ready (per chunk)
    sSHF = sem("sSHF")    # PE: shift matmuls done (2 per chunk)
    sSHB = sem("sSHB")    # DVE: shifted mags copied out of PS0/PS1
    sPM = sem("sPM")      # DVE: P tiles + cast ready (per chunk)
    sSRT = sem("sSRT")    # PE: sort matmul batches (16 per chunk)
    sXSA = sem("sXSA")    # ACT xsT copies (even batches, 8/chunk)
    sXSD = sem("sXSD")    # DVE xsT copies (odd batches, 8/chunk)
    sI16 = sem("sI16")    # DVE: xf consumed (cast+mag) per chunk
    sH1 = sem("sH1")
    sHSB = sem("sHSB")
    sY2 = sem("sY2")
    sY2SB = sem("sY2SB")
    sPEB = sem("sPEB")    # 5 per subchunk: phi1, phi2, sp1, sp2, np-transposes (+1 epilogue)
    sZK = sem("sZK")      # 16 per subchunk: dec hidden matmuls
    sOB = sem("sOB")      # 8 per subchunk: out matmul batches
    sACTB = sem("sACTB")  # 20 per subchunk: z1ab, zsb, s1, npsb, 16 U relus
    sDVEB = sem("sDVEB")  # 1 per subchunk: np consumed + mrow ready
    sOCP = sem("sOCP")    # 1 per out batch copied to outsb
    sND = sem("sND")

    NCONST = len(CONST_SPECS)

    with nc.Block() as block:

        # =================== SYNC: const + x loads + out DMAs ===================
        @block.sync
        def _(e):
            for nm, _, _ in CONST_SPECS:
                e.dma_start(out=C[nm][:], in_=cext[nm][:]).then_inc(sCONST, 16)
            def load_x(c):
                e.dma_start(
                    out=xf[c % 2][:].rearrange("p (j d) -> p j d", d=DIM),
                    in_=x_ext[c * CH_E:(c + 1) * CH_E, :]
                    .rearrange("(j p) d -> p j d", p=128),
                ).then_inc(sX[c % 2], 16)
            load_x(0)
            if n_chunk > 1:
                load_x(1)
            nb = 0
            for c in range(n_chunk):
                if c + 2 < n_chunk:
                    e.wait_ge(sI16, c + 1)
                    load_x(c + 2)
                for ss in range(NSS):
                    row0 = (c * NSS + ss) * SS * MAXN
                    for b in range(8):
                        nb += 1
                        e.wait_ge(sOCP, nb)
                        dst = xr_ext[row0 + 8 * b: row0 + 8 * b + (SS * MAXN - 8 * b), :] \
                            .rearrange("(p t) d -> p (t d)", p=128) if False else \
                            xr_ext[row0:row0 + SS * MAXN, :] \
                            .rearrange("(p t) d -> p t d", p=128)[:, 8 * b:8 * (b + 1), :]
                        e.dma_start(out=dst, in_=outsb[(nb - 1) % 4][:]
                                    .rearrange("p (t d) -> p t d", d=DIM)) \
                            .then_inc(sOUT[(nb - 1) % 4], 16)
            e.wait_ge(sND, 1)
            e.dma_start(out=nd_ext[:].rearrange("(t p) -> t p", t=TCNT),
                        in_=ndi[:TCNT, :]).then_inc(sOUTND, 16)

        # =================== DVE ===================
        @block.vector
        def _(e):
            e.wait_ge(sCONST, 16 * NCONST)
            # one-time memsets (pad slots of cmpb, ones row of U)
            e.memset(cmpb[:], 0.0)
            e.memset(U[0][96:97, :], 1.0)
            e.memset(U[1][96:97, :], 1.0)
            e.drain()
            e.sem_inc(sMS, 1)
            def emit_phase_b(cc):
                bufc = cc % 2
                for ss in range(NSS):
                    sub = cc * NSS + ss
                    e.wait_ge(sPEB, sub * 6 + 6)     # np transposes in PS4
                    e.tensor_copy(npT[:], PS[4][:, 0:128])
                    e.drain()
                    for u in range(8):
                        e.max(mx8[:, 8 * u:8 * u + 8], npT[:, 16 * u:16 * (u + 1)])
                    e.drain()
                    for u in range(8):
                        e.max_index(am8[:, 8 * u:8 * u + 8], mx8[:, 8 * u:8 * u + 8],
                                    npT[:, 16 * u:16 * (u + 1)])
                    e.drain()
                    for u in range(4):
                        e.tensor_copy(ndf[:, sub * 4 + u:sub * 4 + u + 1],
                                      am8[:, 8 * u:8 * u + 1])
                    for u in range(4):
                        e.tensor_copy(ndQ[:, u:u + 1], am8[:, 8 * (u + 4):8 * (u + 4) + 1])
                    e.drain()
                    e.tensor_tensor(out=mrow[:].rearrange("p (cq q) -> p cq q", q=16),
                                    in0=C['iotamod'][:].rearrange("p (cq q) -> p cq q", q=16),
                                    in1=ndQ[:].rearrange("p (cq o) -> p cq o", o=1)
                                    .broadcast_to([128, 4, 16]),
                                    op=ALU.is_lt).then_inc(sDVEB, 1)
                    e.drain()
                    for b in range(8):
                        n_ocp = sub * 8 + b + 1
                        e.wait_ge(sOB, n_ocp)
                        if n_ocp > 4:
                            e.wait_ge(sOUT[(n_ocp - 1) % 4], 16 * ((n_ocp - 1) // 4))
                        pbank = PS[7] if b % 2 == 0 else PS[3]
                        e.tensor_tensor(
                            out=outsb[(n_ocp - 1) % 4][:].rearrange("p (t d) -> p t d", d=DIM),
                            in0=pbank[:].rearrange("p (t d) -> p t d", d=DIM),
                            in1=mrow[:, 8 * b:8 * (b + 1)]
                            .rearrange("p (t o) -> p t o", o=1).broadcast_to([128, 8, DIM]),
                            op=ALU.mult).then_inc(sOCP, 1)

            for c in range(n_chunk):
                buf = c % 2
                # ---- phase A (chunk c) ----
                e.wait_ge(sX[c % 2], 16 * (c // 2 + 1))
                xv = xf[buf][:].rearrange("p (r d) -> p r d", d=DIM)
                for g in range(R // 16):
                    sv = scr[:].rearrange("p (r d) -> p r d", d=DIM)
                    e.tensor_tensor(out=sv, in0=xv[:, 16 * g:16 * (g + 1), :],
                                    in1=C['rankw'][:].rearrange("p (o d) -> p o d", o=1)
                                    .broadcast_to([128, 16, DIM]), op=ALU.mult)
                    e.drain()
                    e.tensor_reduce(out=mag[:, 16 * g:16 * (g + 1)]
                                    .rearrange("p (r o) -> p r o", o=1),
                                    in_=sv, axis=AXX, op=ALU.add)
                    e.drain()
                e.sem_inc(sMAG, 1)
                # cast x to bf16 for the sort matmul lhsT (also frees xf)
                e.tensor_copy(xb16[buf][:], xf[buf][:]).then_inc(sI16, 1)
                # shifted mags arrive from PE in PS3/PS4
                e.wait_ge(sSHF, 2 * c + 2)
                e.tensor_copy(shb[:, 0:7 * R], PS[0][:, 0:7 * R])
                e.tensor_copy(shb[:, 7 * R:14 * R], PS[1][:, 0:7 * R]).then_inc(sSHB, 1)
                e.drain()
                # diffs arrive from PE; fused sign-test + validity mask
                sh3 = shb[:].rearrange("p (s r) -> p s r", r=R)
                cm3 = cmpb[:].rearrange("p (r k) -> p r k", k=16)
                for s in range(1, NPER):
                    e.tensor_scalar(cm3[:, :, 2 * (s - 1)], sh3[:, 2 * (s - 1), :],
                                    0.0, C['MSK'][:, 2 * (s - 1):2 * (s - 1) + 1],
                                    ALU.is_ge, ALU.mult)
                    e.tensor_scalar(cm3[:, :, 2 * (s - 1) + 1],
                                    sh3[:, 2 * (s - 1) + 1, :],
                                    0.0, C['MSK'][:, 2 * (s - 1) + 1:2 * s],
                                    ALU.is_gt, ALU.mult)
                e.drain()
                e.tensor_reduce(out=rank[:].rearrange("p (r o) -> p r o", o=1),
                                in_=cm3, axis=AXX, op=ALU.add)
                e.drain()
                e.tensor_scalar(vv[:], rank[:], C['voff'][:, 0:1], None, ALU.add)
                e.drain()
                # P tiles: one-hot of vv against iota along free (bf16)
                if c > 0:
                    e.wait_ge(sSRT, 16 * c)   # P buffer free (prev sorts done)
                P3 = Pb0[:].rearrange("p (t c2) -> p t c2", c2=128)
                for t in range(R - 1):
                    e.tensor_scalar(P3[:, t, :], C['iota128b'][:],
                                    vv[:, t:t + 1], None, ALU.is_equal)
                e.tensor_scalar(P3[:, R - 1, :], C['iota128b'][:],
                                vv[:, R - 1:R], None, ALU.is_equal).then_inc(sPM, 1)
                if c > 0:
                    emit_phase_b(c - 1)
                # odd sort-batch copies (PS6) -> xsT
                for b2 in range(1, 16, 2):
                    e.wait_ge(sSRT, 16 * c + b2 + 1)
                    e.tensor_copy(xsT[buf][:, 512 * b2:512 * (b2 + 1)],
                                  PS[6][0:64, :]).then_inc(sXSD, 1)
            emit_phase_b(n_chunk - 1)
            # ---- ndec epilogue ----
            e.wait_ge(sPEB, n_sub * 6 + 1)
            e.tensor_copy(ndi[:TCNT, :], PS[4][0:TCNT, 0:128]).then_inc(sND, 1)
            e.drain()

        # =================== ACT ===================
        @block.scalar
        def _(e):
            e.wait_ge(sCONST, 16 * NCONST)
            for c in range(n_chunk):
                buf = c % 2
                for b2 in range(0, 16, 2):
                    e.wait_ge(sSRT, 16 * c + b2 + 1)
                    e.activation(xsT[buf][:, 512 * b2:512 * (b2 + 1)],
                                 PS[5][0:64, :], ACTF.Copy).then_inc(sXSA, 1)
                for i in range(16):
                    e.wait_ge(sH1, 16 * c + i + 1)
                    e.activation(hsT[buf][:, 512 * i:512 * (i + 1)],
                                 PS[i % 2][0:MID_PSI, :], ACTF.Relu,
                                 bias=C['b1'][:, 0:1], scale=1.0).then_inc(sHSB, 1)
                for ss in range(NSS):
                    sub = c * NSS + ss
                    e.wait_ge(sY2, sub + 1)
                    e.activation(y2sb[ss][:], PS[2][:], ACTF.Identity,
                                 bias=C['c2'][:, 0:1], scale=1.0).then_inc(sY2SB, 1)
                    e.wait_ge(sPEB, sub * 6 + 2)
                    e.activation(z1asb[:], PS[5][:], ACTF.Relu,
                                 bias=C['pb1a'][:, 0:1], scale=1.0)
                    e.activation(z1bsb[:], PS[6][0:8, :], ACTF.Relu,
                                 bias=C['pb1b'][:, 0:1], scale=1.0).then_inc(sACTB, 1)
                    e.wait_ge(sPEB, sub * 6 + 3)
                    e.activation(zsb[ss][:], PS[6][:], ACTF.Identity,
                                 bias=C['pb2'][:, 0:1], scale=1.0).then_inc(sACTB, 1)
                    e.wait_ge(sPEB, sub * 6 + 4)
                    e.activation(s1sb[:], PS[5][0:MID_SP, :], ACTF.Relu,
                                 bias=C['spb1'][:, 0:1], scale=1.0).then_inc(sACTB, 1)
                    e.wait_ge(sPEB, sub * 6 + 5)
                    e.activation(npsb[:], PS[5][0:MAXN, :], ACTF.Identity,
                                 bias=C['spb2'][:, 0:1], scale=1.0).then_inc(sACTB, 1)
                    for k in range(MAXN):
                        e.wait_ge(sZK, sub * 16 + k + 1)
                        e.activation(U[ss][0:MID_DEC, :]
                                     .rearrange("p (s k) -> p s k", k=MAXN)[:, :, k],
                                     PS[5 if k % 2 == 0 else 6][0:MID_DEC, :], ACTF.Relu,
                                     bias=C['db1'][:, 0:1], scale=1.0).then_inc(sACTB, 1)

        # =================== PE ===================
        @block.tensor
        def _(e):
            e.wait_ge(sCONST, 16 * NCONST)
            e.wait_ge(sMS, 1)
            def emit_shifts(cs):
                # shift matmuls for chunk cs: 7 down into PS0, 7 up into PS1
                e.wait_ge(sMAG, cs + 1)
                if cs > 0:
                    e.wait_ge(sHSB, 16 * cs)          # PS0/PS1 free (h1 relus done)
                for s in range(1, NPER):
                    mm = e.matmul(PS[0][:, (s - 1) * R:s * R],
                                  C['SH'][:, (2 * (s - 1)) * 128:(2 * (s - 1) + 1) * 128],
                                  mag[:], start=True, stop=True)
                mm.then_inc(sSHF, 1)
                for s in range(1, NPER):
                    mm = e.matmul(PS[1][:, (s - 1) * R:s * R],
                                  C['SH'][:, (2 * (s - 1) + 1) * 128:(2 * s) * 128],
                                  mag[:], start=True, stop=True)
                mm.then_inc(sSHF, 1)
            emit_shifts(0)
            for c in range(n_chunk):
                buf = c % 2
                # sort matmuls: 16 batches of 4 tiles, PS5/PS6 alternate
                e.wait_ge(sPM, c + 1)
                if c > 0:
                    e.wait_ge(sACTB, c * NSS * 20)    # PS5/PS6 free of dec use
                x3 = xb16[buf][:].rearrange("p (t d) -> p t d", d=DIM)
                P3 = Pb0[:].rearrange("p (t c2) -> p t c2", c2=128)
                for b in range(16):
                    if b >= 2:
                        if b % 2 == 0:
                            e.wait_ge(sXSA, c * 8 + b // 2)
                        else:
                            e.wait_ge(sXSD, c * 8 + b // 2)
                    for tt4 in range(4):
                        t = 4 * b + tt4
                        mm = e.matmul(PS[5 if b % 2 == 0 else 6][0:64,
                                      128 * tt4:128 * (tt4 + 1)],
                                      x3[:, t, :], P3[:, t, :], start=True, stop=True)
                    mm.then_inc(sSRT, 1)
                # psi1
                for i in range(16):
                    g = 16 * c + i
                    if i == 0:
                        e.wait_ge(sSHB, c + 1)
                    if i % 2 == 0:
                        e.wait_ge(sXSA, c * 8 + i // 2 + 1)
                    else:
                        e.wait_ge(sXSD, c * 8 + i // 2 + 1)
                    if g >= 2:
                        e.wait_ge(sHSB, g - 1)
                    e.matmul(PS[i % 2][0:MID_PSI, :], C['w1p'][0:DIM, :],
                             xsT[buf][:, 512 * i:512 * (i + 1)],
                             start=True, stop=True).then_inc(sH1, 1)
                if c + 1 < n_chunk:
                    emit_shifts(c + 1)
                for ss in range(NSS):
                    sub = c * NSS + ss
                    # psi2 accumulation (PS2)
                    e.wait_ge(sHSB, 16 * c + 8 * (ss + 1))
                    e.wait_ge(sI16, c + 1)
                    if sub > 0:
                        e.wait_ge(sY2SB, sub)
                    h3 = hsT[buf][:].rearrange("p (s r) -> p s r", r=NPER)
                    for k in range(NPER):
                        mm = e.matmul(PS[2][:], C['W2'][:, 128 * k:128 * (k + 1)],
                                      h3[:, ss * SS:(ss + 1) * SS, k],
                                      start=(k == 0), stop=(k == NPER - 1))
                    mm.then_inc(sY2, 1)
                    # phi1 -> PS5 (z1a), PS6[0:8] (z1b)
                    e.wait_ge(sY2SB, sub + 1)
                    if sub > 0:
                        e.wait_ge(sACTB, sub * 20)       # prev sub fully drained
                        e.wait_ge(sOCP, sub * 8)         # PS3/PS7 prev out copied
                    e.matmul(PS[5][:], C['pw1a'][:], y2sb[ss][:], start=True, stop=True) \
                        .then_inc(sPEB, 1)
                    e.matmul(PS[6][0:8, :], C['pw1b'][:], y2sb[ss][:], start=True, stop=True) \
                        .then_inc(sPEB, 1)               # P1a/P1b
                    # phi2 -> PS6
                    e.wait_ge(sACTB, sub * 20 + 1)
                    e.matmul(PS[6][:], C['pw2a'][:], z1asb[:], start=True, stop=False)
                    e.matmul(PS[6][:], C['pw2b'][:], z1bsb[:], start=False, stop=True) \
                        .then_inc(sPEB, 1)               # P2
                    # sp1 -> PS5
                    e.wait_ge(sACTB, sub * 20 + 2)
                    e.matmul(PS[5][0:MID_SP, :], C['spw1'][:], zsb[ss][:],
                             start=True, stop=True).then_inc(sPEB, 1)   # P3
                    # sp2 -> PS5[0:16]
                    e.wait_ge(sACTB, sub * 20 + 3)
                    e.matmul(PS[5][0:MAXN, :], C['spw2'][:], s1sb[:],
                             start=True, stop=True).then_inc(sPEB, 1)   # P4
                    # np transposes -> PS4[:, 0:128]
                    e.wait_ge(sACTB, sub * 20 + 4)
                    if sub > 0:
                        e.wait_ge(sDVEB, sub)
                    for tt in range(4):
                        e.transpose(PS[4][:, 16 * tt:16 * (tt + 1)],
                                    npsb[:, 128 * tt:128 * (tt + 1)], C['eye16'][:])
                    for tt in range(4):
                        tp = e.transpose(PS[4][:, 64 + 16 * tt:64 + 16 * (tt + 1)],
                                         npsb[:].rearrange("p (j q) -> p q j", q=4)[:, tt, :],
                                         C['eye16'][:])
                    tp.then_inc(sPEB, 1)                 # P5
                    # dec hidden: 16 matmuls, PS5 (even k) / PS6 (odd k)
                    for k in range(MAXN):
                        if k == 0:
                            e.wait_ge(sACTB, sub * 20 + 4)
                        elif k == 1:
                            e.wait_ge(sACTB, sub * 20 + 2)
                        else:
                            e.wait_ge(sACTB, sub * 20 + 4 + (k - 1))
                        e.matmul(PS[5 if k % 2 == 0 else 6][0:MID_DEC, :],
                                 C['D1'][:, MID_DEC * k:MID_DEC * (k + 1)], zsb[ss][:],
                                 start=True, stop=True).then_inc(sZK, 1)
                    # out matmuls: 8 batches of 8 tiles
                    e.wait_ge(sACTB, sub * 20 + 20)
                    for b in range(8):
                        gb = sub * 8 + b
                        if gb >= 2:
                            e.wait_ge(sOCP, gb - 1)
                        pbank = PS[7] if b % 2 == 0 else PS[3]
                        Uv = U[ss][0:MID_DEC + 1, :].rearrange("p (s t) -> p t s", t=64)
                        for t in range(8):
                            mm = e.matmul(pbank[:, 64 * t:64 * (t + 1)],
                                          Uv[:, 8 * b + t, :], C['w2e'][:],
                                          start=True, stop=True)
                        mm.then_inc(sOB, 1)
            # ndec epilogue transpose -> PS4
            e.wait_ge(sDVEB, n_sub)
            e.transpose(PS[4][0:TCNT, 0:128], ndf[:, 0:TCNT], C['eye128'][:]) \
                .then_inc(sPEB, 1)

    es.close()
    return nc


# ---------------------------------------------------------------------------
# Host entry point
# ---------------------------------------------------------------------------
_CACHE = {}


def _get_nc():
    if 'nc' not in _CACHE:
        import concourse.bacc as bacc
        nc = bacc.Bacc()
        build(nc)
        nc.compile()
        _CACHE['nc'] = nc
    return _CACHE['nc']


def kernel(x, batch, params):
    import ml_dtypes
    from concourse.bass_utils import run_bass_kernel_spmd

    x = np.ascontiguousarray(np.asarray(x, dtype=np.float32))
    consts = host_prep(params)

    nc = _get_nc()
    in_maps = []
    for core in range(NCORES):
        m = {'x': x[core * NC:(core + 1) * NC]}
        for nm, shp, dt in CONST_SPECS:
            v = consts[nm]
            assert list(v.shape) == shp, (nm, v.shape, shp)
            m[nm] = v
        in_maps.append(m)

    res = run_bass_kernel_spmd(nc, in_maps, list(range(NCORES)))
    xr = np.empty((B, MAXN, DIM), dtype=np.float32)
    nd = np.empty((B,), dtype=np.int32)
    for core in range(NCORES):
        r = res.results[core]
        xr[core * BC:(core + 1) * BC] = r['xr'].reshape(BC, MAXN, DIM)
        nd[core * BC:(core + 1) * BC] = r['nd']
    return xr, nd
